# revision 5
# baseline (speedup 1.0000x reference)
"""JointAngleLoss Trainium2 kernel (8-core data-parallel), v3.

Input : pose23d_pred [524288, 21, 3] float32
Output: scalar float32 loss (matches reference.reference)

Strategy: pure data-parallel over the batch dim; each of 8 NeuronCores handles
65536 rows. Host pre-permutes the input into a per-partition slot layout
J[c][jj][f][k] (duplicating the 4 shared joints: 75 values per row) and casts
to fp16 (loss tolerance 2e-2; measured fp16 input-cast error ~1e-6 relative),
so every device-side vector operand is a contiguous fp16 slice (DVE 2x_1P
packed mode) and DMA bytes are halved vs fp32.

Device pipeline per group (K=256 rows/slot, G=2 groups):
  DMA fp16 -> DVE bones(1 instr, 3D AP)/crosses(m1,m2: 6; rc in-place: 1)/
  pc,red(2)/pp(1 merged, broadcast operand)/vsum(2)
  -> ACT relu(-v), square with fp32 accum_out (overlapped)
  -> PE ones-matmul reduces coplanarity products into PSUM fp32.
Host sums the per-core partials in float64.
"""

import sys

for _p in ("/opt/trn_rl_repo", "/root/.axon_site/_ro/trn_rl_repo"):
    if _p not in sys.path:
        sys.path.append(_p)

import numpy as np

import concourse.bacc as bacc
import concourse.mybir as mybir
from concourse import tile
from concourse.bass_utils import run_bass_kernel_spmd
from contextlib import ExitStack

N_CORES = 8
P = 128          # SBUF partitions
B_FULL = 524288  # total batch
ROW = 75         # 3 comps * 5 joint-slots * 5 fingers (shared joints duplicated)
DEF_K = 256

F16 = mybir.dt.float16
F32 = mybir.dt.float32


def build_bass(rows_per_core: int, K: int, reps: int = 1, hw_loop: int = 1,
               pool_units: int = 0, merged_pp: bool = True):
    """rows_per_core = P * K * G.  K = rows per partition slot per group.

    reps>1 unrolls the compute (timing); hw_loop>1 wraps it in a device-side
    For_i (timing; outputs = last iteration's = one correct pass).
    pool_units in {0,3,6,10} moves part of the elementwise work to GpSimd.
    """
    assert rows_per_core % (P * K) == 0
    G = rows_per_core // (P * K)
    FK = ROW * K          # fp16 elems per partition per group (75*K)
    CJ = 25 * K           # joint elems per component (5jj*5f*K)
    CB = 20 * K           # bone elems per component  (4jj*5f*K)
    S5 = 5 * K            # one [f][k] slab
    NR = 9 * S5           # 3c * 3q * S5: m1/m2/rot elems per partition
    NCOP = 3 * S5         # coplane products per partition
    NV = 2 * S5           # v values per partition

    nc = bacc.Bacc("TRN2", target_bir_lowering=False, debug=False)

    x = nc.dram_tensor("x", [G, P, FK], F16, kind="ExternalInput")
    cop_out = nc.dram_tensor("cop_out", [1, NCOP], F32, kind="ExternalOutput")
    mask_out = nc.dram_tensor("mask_out", [P, G * reps], F32, kind="ExternalOutput")

    with tile.TileContext(nc) as tc, ExitStack() as ctx:
        xpool = ctx.enter_context(tc.tile_pool(name="xpool", bufs=2))
        bpool = ctx.enter_context(tc.tile_pool(name="bpool", bufs=1))
        mpool = ctx.enter_context(tc.tile_pool(name="mpool", bufs=1))
        vpool = ctx.enter_context(tc.tile_pool(name="vpool", bufs=1))
        spool = ctx.enter_context(tc.tile_pool(name="spool", bufs=1))
        psum = ctx.enter_context(tc.tile_pool(name="psum", bufs=1, space="PSUM"))

        ones = spool.tile([P, 1], F16)
        nc.gpsimd.memset(ones[:], 1.0)
        acc = spool.tile([P, G * reps], F32)
        psum_cop = psum.tile([1, NCOP], F32)

        n_chunks = (NCOP + 511) // 512

        loop_cm = tc.For_i(0, hw_loop, 1) if hw_loop > 1 else None
        if loop_cm is not None:
            loop_cm.__enter__()

        for rep in range(reps):
            for g in range(G):
                first = rep == 0 and g == 0
                last = rep == reps - 1 and g == G - 1

                # ---- load fp16 (split in half for earlier compute start)
                xh = xpool.tile([P, FK], F16)
                half = FK // 2
                for h in range(2):
                    sl = slice(h * half, (h + 1) * half)
                    nc.sync.dma_start(xh[:, sl], x.ap()[g][:, sl])

                # ---- bones: B[c][jj][f][k] = J[c][jj+1][f][k]-J[c][jj][f][k]
                bones = bpool.tile([P, 3 * CB], F16)
                xv = xh[:].rearrange("p (c n) -> p c n", c=3)
                bv = bones[:].rearrange("p (c n) -> p c n", c=3)
                if pool_units >= 10:
                    nc.vector.tensor_sub(bv[:, 0:2], xv[:, 0:2, S5 : S5 + CB],
                                         xv[:, 0:2, 0:CB])
                    nc.gpsimd.tensor_sub(bones[:, 2 * CB : 3 * CB],
                                         xh[:, 2 * CJ + S5 : 2 * CJ + CJ],
                                         xh[:, 2 * CJ : 2 * CJ + CB])
                else:
                    nc.vector.tensor_sub(bv, xv[:, :, S5 : S5 + CB], xv[:, :, 0:CB])

                # ---- cross products, c-major [c][q][f][k] -------------------
                # rot[c][q] = B_{c1}[q+1]*B_{c2}[q] - B_{c2}[q+1]*B_{c1}[q]
                m1 = mpool.tile([P, NR], F16, tag="m1")
                m2 = mpool.tile([P, NR], F16, tag="m2")
                for c in range(3):
                    c1, c2 = (c + 1) % 3, (c + 2) % 3
                    e1 = nc.gpsimd if (pool_units >= 6 and c == 2) else nc.vector
                    e2 = nc.gpsimd if (pool_units >= 3 and c == 2) else nc.vector
                    e1.tensor_mul(
                        m1[:, c * NCOP : (c + 1) * NCOP],
                        bones[:, c1 * CB + S5 : c1 * CB + CB],
                        bones[:, c2 * CB : c2 * CB + NCOP])
                    e2.tensor_mul(
                        m2[:, c * NCOP : (c + 1) * NCOP],
                        bones[:, c2 * CB + S5 : c2 * CB + CB],
                        bones[:, c1 * CB : c1 * CB + NCOP])
                rot = m1  # rc computed in place: rot = m1 - m2
                nc.vector.tensor_sub(rot[:], m1[:], m2[:])

                rv = rot[:].rearrange("p (c n) -> p c n", c=3)  # [P,3,3*S5]

                def qb(q):  # q-th cross block for each c: [P, 3, S5]
                    return rv[:, :, q * S5 : (q + 1) * S5]

                # ---- coplane products: (palm + mid)_c * b4_c ---------------
                pc = vpool.tile([P, NCOP], F16, tag="pc")
                red = vpool.tile([P, NCOP], F16, tag="red")
                ncv = lambda t: t[:].rearrange("p (c n) -> p c n", c=3)
                nc.vector.tensor_add(ncv(pc), qb(0), qb(1))
                nc.vector.tensor_mul(ncv(red), ncv(pc), bv[:, :, 3 * S5 : 4 * S5])

                # ---- v1 = tip.mid, v2 = palm.mid; pp[c] = [v2_c | v1_c] ----
                pp = vpool.tile([P, 6 * S5], F16, tag="pp")
                ppv = pp[:].rearrange("p (c w n) -> p w c n", c=3, w=2)
                if merged_pp:
                    rq = rot[:].rearrange("p (c q n) -> p q c n", c=3, q=3)
                    nc.vector.tensor_mul(
                        ppv, rq[:, 0:3:2],
                        rq[:, 1:2].broadcast_to([P, 2, 3, S5]))
                else:
                    nc.vector.tensor_mul(ppv[:, 1], qb(2), qb(1))
                    nc.vector.tensor_mul(ppv[:, 0], qb(0), qb(1))
                vs = vpool.tile([P, NV], F16, tag="vs")
                v = vpool.tile([P, NV], F16, tag="v")
                nc.vector.tensor_add(vs[:], pp[:, 0:NV], pp[:, NV : 2 * NV])
                nc.vector.tensor_add(v[:], vs[:], pp[:, 2 * NV : 3 * NV])

                # ---- masked squares on ACT: sum(relu(-v)^2) -> acc ----------
                nc.scalar.activation(vs[:], v[:], mybir.ActivationFunctionType.Relu,
                                     scale=-1.0)
                nc.scalar.activation(v[:], vs[:],
                                     mybir.ActivationFunctionType.Square,
                                     accum_out=acc[:, rep * G + g : rep * G + g + 1])

                # ---- PE reduction of coplane products over partitions -------
                for i in range(n_chunks):
                    lo = 512 * i
                    hi = min(NCOP, lo + 512)
                    nc.tensor.matmul(psum_cop[:, lo:hi], ones[:], red[:, lo:hi],
                                     start=first, stop=last)

        if loop_cm is not None:
            loop_cm.__exit__(None, None, None)

        # ---- epilogue: PSUM -> SBUF -> DRAM ---------------------------------
        cop_sb = spool.tile([1, NCOP], F32)
        nc.scalar.copy(cop_sb[:], psum_cop[:])
        nc.sync.dma_start(cop_out.ap(), cop_sb[:])
        nc.sync.dma_start(mask_out.ap(), acc[:])

    nc.compile()
    return nc, G


def host_planarize(x: np.ndarray, n_cores: int, K: int) -> np.ndarray:
    """[B,21,3] f32 -> [cores, G, P, 75K] f16: slot layout [c][jj:5][f:5][k]."""
    B = x.shape[0]
    R = B // n_cores
    G = R // (P * K)
    xr = x.reshape(n_cores, G, P, K, 21, 3)
    jidx = (np.arange(5) * 4)[:, None] + np.arange(5)[None, :]  # [f, jj]
    xj = xr[:, :, :, :, jidx, :]                 # [cores,G,P,K,f,jj,3]
    xp = xj.transpose(0, 1, 2, 6, 5, 4, 3)       # [cores,G,P,c,jj,f,K]
    out = np.empty((n_cores, G, P, ROW * K), dtype=np.float16)
    np.copyto(out.reshape(xp.shape), xp)
    return out


_CACHE = {}


def _get_nc(rows_per_core: int, K: int):
    key = (rows_per_core, K)
    if key not in _CACHE:
        _CACHE[key] = build_bass(rows_per_core, K)
    return _CACHE[key]


def kernel(pose23d_pred: np.ndarray) -> np.ndarray:
    x = np.asarray(pose23d_pred, dtype=np.float32)
    assert x.shape == (B_FULL, 21, 3), x.shape
    K = DEF_K
    R = B_FULL // N_CORES
    nc, G = _get_nc(R, K)
    xp = host_planarize(x, N_CORES, K)
    in_maps = [{"x": xp[i]} for i in range(N_CORES)]
    res = run_bass_kernel_spmd(nc, in_maps, list(range(N_CORES)))
    total = 0.0
    for r in res.results:
        total += r["cop_out"].astype(np.float64).sum()
        total += r["mask_out"].astype(np.float64).sum()
    return np.float32(total)


# revision 7
# speedup vs baseline: 1.0779x; 1.0779x over previous
"""JointAngleLoss Trainium2 kernel (8-core data-parallel), v4.

Input : pose23d_pred [524288, 21, 3] float32
Output: scalar float32 loss (matches reference.reference)

Strategy: pure data-parallel over the batch dim; each of 8 NeuronCores handles
65536 rows. Host pre-permutes the input into a per-partition slot layout
J[c][jj][f][k] (duplicating the 4 shared joints: 75 values per row) and casts
to fp16 (loss tolerance 2e-2; measured fp16 input-cast error ~1e-6 relative),
so every device-side vector operand is a contiguous fp16 slice (DVE 2x_1P
packed mode) and DMA bytes are halved vs fp32.

Groups are processed in PAIRS with their DVE instruction streams interleaved:
adjacent instructions come from independent groups, hiding the DVE pipe-DRAIN
that back-to-back dependent ops would expose.

Per group: DMA fp16 -> DVE bones(1x 3D AP)/crosses(3+3, rc)/pc,red/pp(merged,
broadcast operand)/vsums -> ACT relu(-v)+square with fp32 accum_out
(overlapped) -> PE ones-matmul reduces coplanarity products into PSUM fp32.
Host sums the per-core partials in float64.
"""

import sys

for _p in ("/opt/trn_rl_repo", "/root/.axon_site/_ro/trn_rl_repo"):
    if _p not in sys.path:
        sys.path.append(_p)

import numpy as np

import concourse.bacc as bacc
import concourse.mybir as mybir
from concourse import tile
from concourse.bass_utils import run_bass_kernel_spmd
from contextlib import ExitStack

N_CORES = 8
P = 128          # SBUF partitions
B_FULL = 524288  # total batch
ROW = 75         # 3 comps * 5 joint-slots * 5 fingers (shared joints duplicated)
DEF_K = 128

F16 = mybir.dt.float16
F32 = mybir.dt.float32


def build_bass(rows_per_core: int, K: int, reps: int = 1, hw_loop: int = 1,
               pool_units: int = 0):
    """rows_per_core = P * K * G.  K = rows per partition slot per group.

    reps>1 unrolls the compute (timing); hw_loop>1 wraps it in a device-side
    For_i (timing; outputs = last iteration's = one correct pass).
    pool_units in {0,3,6} moves part of the elementwise work to GpSimd.
    """
    assert rows_per_core % (P * K * 2) == 0
    G = rows_per_core // (P * K)
    FK = ROW * K          # fp16 elems per partition per group (75*K)
    CJ = 25 * K           # joint elems per component (5jj*5f*K)
    CB = 20 * K           # bone elems per component  (4jj*5f*K)
    S5 = 5 * K            # one [f][k] slab
    NR = 9 * S5           # 3c * 3q * S5: m1/m2/rot elems per partition
    NCOP = 3 * S5         # coplane products per partition
    NV = 2 * S5           # v values per partition

    nc = bacc.Bacc("TRN2", target_bir_lowering=False, debug=False)

    x = nc.dram_tensor("x", [G, P, FK], F16, kind="ExternalInput")
    cop_out = nc.dram_tensor("cop_out", [1, NCOP], F32, kind="ExternalOutput")
    mask_out = nc.dram_tensor("mask_out", [P, G * reps], F32, kind="ExternalOutput")

    with tile.TileContext(nc) as tc, ExitStack() as ctx:
        xpool = ctx.enter_context(tc.tile_pool(name="xpool", bufs=2))
        bpool = ctx.enter_context(tc.tile_pool(name="bpool", bufs=2))
        mpool = ctx.enter_context(tc.tile_pool(name="mpool", bufs=2))
        vpool = ctx.enter_context(tc.tile_pool(name="vpool", bufs=2))
        spool = ctx.enter_context(tc.tile_pool(name="spool", bufs=1))
        psum = ctx.enter_context(tc.tile_pool(name="psum", bufs=1, space="PSUM"))

        ones = spool.tile([P, 1], F16)
        nc.gpsimd.memset(ones[:], 1.0)
        acc = spool.tile([P, G * reps], F32)
        psum_cop = psum.tile([1, NCOP], F32)

        n_chunks = (NCOP + 511) // 512
        c3 = lambda ap: ap.rearrange("p (c n) -> p c n", c=3)

        loop_cm = tc.For_i(0, hw_loop, 1) if hw_loop > 1 else None
        if loop_cm is not None:
            loop_cm.__enter__()

        for rep in range(reps):
            for g0 in range(0, G, 2):
                pair = (g0, g0 + 1)
                st = [{}, {}]  # per-group tile state

                for i, g in enumerate(pair):
                    xh = xpool.tile([P, FK], F16, tag="xh", name="xh")
                    half = FK // 2
                    for h in range(2):
                        sl = slice(h * half, (h + 1) * half)
                        nc.sync.dma_start(xh[:, sl], x.ap()[g][:, sl])
                    st[i]["xh"] = xh

                # ---- bones: B[c][jj][f][k] = J[c][jj+1] - J[c][jj], 1 instr
                for i in range(2):
                    bones = bpool.tile([P, 3 * CB], F16, tag="bones", name="bones")
                    xv = c3(st[i]["xh"][:])
                    bv = c3(bones[:])
                    nc.vector.tensor_sub(bv, xv[:, :, S5 : S5 + CB], xv[:, :, 0:CB])
                    st[i]["bones"] = bones

                # ---- cross products, c-major [c][q][f][k] -------------------
                # rot[c][q] = B_{c1}[q+1]*B_{c2}[q] - B_{c2}[q+1]*B_{c1}[q]
                for i in range(2):
                    st[i]["m1"] = mpool.tile([P, NR], F16, tag="m1", name="m1")
                    st[i]["m2"] = mpool.tile([P, NR], F16, tag="m2", name="m2")
                    st[i]["rot"] = mpool.tile([P, NR], F16, tag="rot", name="rot")
                for c in range(3):
                    c1, c2 = (c + 1) % 3, (c + 2) % 3
                    for which, a_off, b_off in (
                        ("m1", c1 * CB + S5, c2 * CB),
                        ("m2", c2 * CB + S5, c1 * CB),
                    ):
                        for i in range(2):
                            bones = st[i]["bones"]
                            eng = nc.vector
                            if pool_units >= 3 and c == 2 and which == "m2":
                                eng = nc.gpsimd
                            if pool_units >= 6 and c == 2 and which == "m1":
                                eng = nc.gpsimd
                            eng.tensor_mul(
                                st[i][which][:, c * NCOP : (c + 1) * NCOP],
                                bones[:, a_off : a_off + NCOP],
                                bones[:, b_off : b_off + NCOP])
                for i in range(2):
                    nc.vector.tensor_sub(st[i]["rot"][:], st[i]["m1"][:],
                                         st[i]["m2"][:])

                # ---- coplane products: (palm + mid)_c * b4_c ---------------
                for i in range(2):
                    st[i]["pc"] = vpool.tile([P, NCOP], F16, tag="pc", name="pc")
                    st[i]["red"] = vpool.tile([P, NCOP], F16, tag="red", name="red")
                for i in range(2):
                    rv = c3(st[i]["rot"][:])
                    nc.vector.tensor_add(c3(st[i]["pc"][:]),
                                         rv[:, :, 0:S5], rv[:, :, S5 : 2 * S5])
                for i in range(2):
                    bv = c3(st[i]["bones"][:])
                    nc.vector.tensor_mul(c3(st[i]["red"][:]), c3(st[i]["pc"][:]),
                                         bv[:, :, 3 * S5 : 4 * S5])

                # ---- v1 = tip.mid, v2 = palm.mid; pp[c] = [v2_c | v1_c] ----
                for i in range(2):
                    st[i]["pp"] = vpool.tile([P, 6 * S5], F16, tag="pp", name="pp")
                for i in range(2):
                    rq = st[i]["rot"][:].rearrange("p (c q n) -> p q c n", c=3, q=3)
                    ppv = st[i]["pp"][:].rearrange("p (c w n) -> p w c n", c=3, w=2)
                    nc.vector.tensor_mul(ppv, rq[:, 0:3:2],
                                         rq[:, 1:2].broadcast_to([P, 2, 3, S5]))
                for i in range(2):
                    st[i]["vs"] = vpool.tile([P, NV], F16, tag="vs", name="vs")
                    st[i]["v"] = vpool.tile([P, NV], F16, tag="v", name="v")
                for i in range(2):
                    pp = st[i]["pp"]
                    nc.vector.tensor_add(st[i]["vs"][:], pp[:, 0:NV],
                                         pp[:, NV : 2 * NV])
                for i in range(2):
                    nc.vector.tensor_add(st[i]["v"][:], st[i]["vs"][:],
                                         st[i]["pp"][:, 2 * NV : 3 * NV])

                # ---- masked squares on ACT: sum(relu(-v)^2) -> acc ----------
                for i, g in enumerate(pair):
                    nc.scalar.activation(st[i]["vs"][:], st[i]["v"][:],
                                         mybir.ActivationFunctionType.Relu,
                                         scale=-1.0)
                    nc.scalar.activation(st[i]["v"][:], st[i]["vs"][:],
                                         mybir.ActivationFunctionType.Square,
                                         accum_out=acc[:, rep * G + g : rep * G + g + 1])

                # ---- PE reduction of coplane products over partitions -------
                for i, g in enumerate(pair):
                    first = rep == 0 and g == 0
                    last = rep == reps - 1 and g == G - 1
                    for j in range(n_chunks):
                        lo = 512 * j
                        hi = min(NCOP, lo + 512)
                        nc.tensor.matmul(psum_cop[:, lo:hi], ones[:],
                                         st[i]["red"][:, lo:hi],
                                         start=first, stop=last)

        if loop_cm is not None:
            loop_cm.__exit__(None, None, None)

        # ---- epilogue: PSUM -> SBUF -> DRAM ---------------------------------
        cop_sb = spool.tile([1, NCOP], F32)
        nc.scalar.copy(cop_sb[:], psum_cop[:])
        nc.sync.dma_start(cop_out.ap(), cop_sb[:])
        nc.sync.dma_start(mask_out.ap(), acc[:])

    nc.compile()
    return nc, G


def host_planarize(x: np.ndarray, n_cores: int, K: int) -> np.ndarray:
    """[B,21,3] f32 -> [cores, G, P, 75K] f16: slot layout [c][jj:5][f:5][k]."""
    B = x.shape[0]
    R = B // n_cores
    G = R // (P * K)
    xr = x.reshape(n_cores, G, P, K, 21, 3)
    jidx = (np.arange(5) * 4)[:, None] + np.arange(5)[None, :]  # [f, jj]
    xj = xr[:, :, :, :, jidx, :]                 # [cores,G,P,K,f,jj,3]
    xp = xj.transpose(0, 1, 2, 6, 5, 4, 3)       # [cores,G,P,c,jj,f,K]
    out = np.empty((n_cores, G, P, ROW * K), dtype=np.float16)
    np.copyto(out.reshape(xp.shape), xp)
    return out


_CACHE = {}


def _get_nc(rows_per_core: int, K: int):
    key = (rows_per_core, K)
    if key not in _CACHE:
        _CACHE[key] = build_bass(rows_per_core, K)
    return _CACHE[key]


def kernel(pose23d_pred: np.ndarray) -> np.ndarray:
    x = np.asarray(pose23d_pred, dtype=np.float32)
    assert x.shape == (B_FULL, 21, 3), x.shape
    K = DEF_K
    R = B_FULL // N_CORES
    nc, G = _get_nc(R, K)
    xp = host_planarize(x, N_CORES, K)
    in_maps = [{"x": xp[i]} for i in range(N_CORES)]
    res = run_bass_kernel_spmd(nc, in_maps, list(range(N_CORES)))
    total = 0.0
    for r in res.results:
        total += r["cop_out"].astype(np.float64).sum()
        total += r["mask_out"].astype(np.float64).sum()
    return np.float32(total)
